# revision 2
# baseline (speedup 1.0000x reference)
"""LSTM forward on 8 Trainium2 NeuronCores (Bass/Tile).

Problem: B=64, S=512, D=H=1024, gates (i, f, o, c_hat):
    x_proj = einsum('bsd,ghd->sbgh', input, Wx)
    per step: g = x_proj[t] + h @ Wh[g]^T + b ; standard LSTM cell
    returns (output[B,S,H], (h_last, c_last))

Sharding: tensor-parallel over H. Core j owns hidden lanes [128j, 128j+128)
for all 4 gates. Per step, each core computes its 128 lanes of h_t and the
cores AllGather h_t (bf16, 16KB/core) for the next step's recurrent matmul.
The input projection (parallel over lanes too: each core needs only its own
Wx rows) is interleaved into the recurrence to fill the PE during AG waits.

Layout choice: the recurrent matmul puts gate-lanes on PSUM partitions
(lhsT = Wh^T chunk [128 h-lanes, 128 gate-lanes], rhs = gathered h^T chunk
[128 h-lanes, 64 batch]), so the whole epilogue runs on full 128 partitions
([lane, batch] tiles) and h_t^T [128, 64] comes out already in the layout the
AllGather needs — no transposes anywhere on the critical path.
"""
import numpy as np
import ml_dtypes
from contextlib import ExitStack

import concourse.bass as bass
import concourse.mybir as mybir
import concourse.tile as tile
from concourse import bacc

BF16 = mybir.dt.bfloat16
F32 = mybir.dt.float32

B, S, D, H = 64, 512, 1024, 1024
N_CORES = 8
LPC = H // N_CORES          # 128 lanes per core
KC = D // 128               # 8 contraction chunks
ROW_TILE = 512              # phase-1 rows (s,b) per tile = 8 steps
N_TILES = S * B // ROW_TILE  # 64
PREF_TILES = 16             # phase-1 tiles emitted before the recurrence
CADENCE = 9                 # then one tile every CADENCE steps

_cache = {}


def _build(s_steps=S):
    nt_total = s_steps * B // ROW_TILE
    pref = min(PREF_TILES, nt_total)
    nc = bacc.Bacc("TRN2", target_bir_lowering=False)

    inT = nc.dram_tensor("inT", [D, s_steps * B], BF16, kind="ExternalInput")
    wxT = nc.dram_tensor("wxT", [D, 4 * LPC], BF16, kind="ExternalInput")
    whT = nc.dram_tensor("whT", [H, 4 * LPC], BF16, kind="ExternalInput")
    bT = nc.dram_tensor("bT", [LPC, 4], F32, kind="ExternalInput")
    out_loc = nc.dram_tensor("out_loc", [s_steps, LPC, B], BF16, kind="ExternalOutput")
    c_last = nc.dram_tensor("c_last", [LPC, B], F32, kind="ExternalOutput")

    with ExitStack() as ctx:
        tc = ctx.enter_context(tile.TileContext(nc))
        singles = ctx.enter_context(tc.tile_pool(name="singles", bufs=1))
        p1_in = ctx.enter_context(tc.tile_pool(name="p1_in", bufs=2))
        p1_ps = ctx.enter_context(tc.tile_pool(name="p1_ps", bufs=2, space="PSUM"))
        p1_xp = ctx.enter_context(tc.tile_pool(name="p1_xp", bufs=3))
        p2_ps = ctx.enter_context(tc.tile_pool(name="p2_ps", bufs=2, space="PSUM"))
        p2_sb = ctx.enter_context(tc.tile_pool(name="p2_sb", bufs=2))
        p2_xp = ctx.enter_context(tc.tile_pool(name="p2_xp", bufs=4))
        dram = ctx.enter_context(tc.tile_pool(name="dram", bufs=1, space="DRAM"))

        # resident weights / bias
        wx_sb = singles.tile([128, KC * 4 * LPC], BF16, name="wx_sb")
        wh_sb = singles.tile([128, KC * 4 * LPC], BF16, name="wh_sb")
        bt_sb = singles.tile([LPC, 4], F32, name="bt_sb")
        nc.sync.dma_start(out=wx_sb[:].rearrange("p (k m) -> p k m", k=KC),
                          in_=wxT[:, :].rearrange("(k p) m -> p k m", p=128))
        nc.sync.dma_start(out=wh_sb[:].rearrange("p (k m) -> p k m", k=KC),
                          in_=whT[:, :].rearrange("(k p) m -> p k m", p=128))
        nc.sync.dma_start(out=bt_sb[:], in_=bT[:, :])

        # recurrence state
        hall = [p2_sb.tile([128, N_CORES * B], BF16, name=f"hall{i}", tag=f"hall{i}")
                for i in range(2)]
        cbuf = [p2_sb.tile([128, B], F32, name=f"cbuf{i}", tag=f"cbuf{i}")
                for i in range(2)]

        # internal DRAM
        xp_d = dram.tile([s_steps, 128, 4 * B], F32, name="xp_d")
        inb = [dram.tile([128, B], BF16, name=f"inb{i}", tag=f"inb{i}")
               for i in range(2)]
        outb = [dram.tile([N_CORES * 128, B], BF16, name=f"outb{t}",
                          addr_space="Shared") for t in range(s_steps - 1)]

        def emit_p1_tile(nt):
            """Input projection for rows [512*nt, 512*nt+512) = steps 8nt..8nt+8."""
            r0 = ROW_TILE * nt
            in_tiles = []
            for k in range(KC):
                it = p1_in.tile([128, ROW_TILE], BF16, name=f"it{nt}_{k}", tag=f"it{k}")
                nc.sync.dma_start(out=it[:], in_=inT[128 * k:128 * (k + 1), r0:r0 + ROW_TILE])
                in_tiles.append(it)
            for g in range(4):
                ps = p1_ps.tile([128, ROW_TILE], F32, name=f"ps1_{nt}_{g}", tag="ps1")
                for k in range(KC):
                    nc.tensor.matmul(ps[:], lhsT=wx_sb[:, k * 512 + g * 128:k * 512 + (g + 1) * 128],
                                     rhs=in_tiles[k][:], start=(k == 0), stop=(k == KC - 1))
                xs = p1_xp.tile([128, ROW_TILE], F32, name=f"xs{nt}_{g}", tag="xs")
                nc.vector.tensor_scalar_add(out=xs[:], in0=ps[:], scalar1=bt_sb[:, g:g + 1])
                nc.sync.dma_start(
                    out=xp_d[8 * nt:8 * nt + 8, :, B * g:B * (g + 1)].rearrange("t p b -> p t b"),
                    in_=xs[:].rearrange("p (t b) -> p t b", t=8))

        def emit_step(t):
            xp2 = p2_xp.tile([128, 4 * B], F32, name=f"xp2_{t}", tag="xp2")
            nc.sync.dma_start(out=xp2[:], in_=xp_d[t, :, :])
            gbuf = p2_sb.tile([128, 4 * B], F32, name=f"g{t}", tag="gbuf")
            if t == 0:
                nc.vector.tensor_copy(out=gbuf[:], in_=xp2[:])
            else:
                ps = p2_ps.tile([128, 4 * B], F32, name=f"ps2_{t}", tag="ps2")
                h_in = hall[t % 2]
                for g in range(4):
                    for k in range(KC):
                        nc.tensor.matmul(
                            ps[:, B * g:B * (g + 1)],
                            lhsT=wh_sb[:, k * 512 + g * 128:k * 512 + (g + 1) * 128],
                            rhs=h_in[:, B * k:B * (k + 1)],
                            start=(k == 0), stop=(k == KC - 1))
                nc.vector.tensor_add(out=gbuf[:], in0=ps[:], in1=xp2[:])
            # gates: i=[0:64] f=[64:128] o=[128:192] chat=[192:256]
            sig = p2_sb.tile([128, 3 * B], F32, name=f"sig{t}", tag="sig")
            cha = p2_sb.tile([128, B], F32, name=f"cha{t}", tag="cha")
            nc.scalar.activation(out=sig[:], in_=gbuf[:, 0:3 * B],
                                 func=mybir.ActivationFunctionType.Sigmoid)
            nc.scalar.activation(out=cha[:], in_=gbuf[:, 3 * B:4 * B],
                                 func=mybir.ActivationFunctionType.Tanh)
            c_old, c_new = cbuf[t % 2], cbuf[(t + 1) % 2]
            t1 = p2_sb.tile([128, B], F32, name=f"t1_{t}", tag="t1")
            t2 = p2_sb.tile([128, B], F32, name=f"t2_{t}", tag="t2")
            if t == 0:
                # c_old = 0: c_new = i * chat
                nc.vector.tensor_mul(out=c_new[:], in0=sig[:, 0:B], in1=cha[:])
            else:
                nc.vector.tensor_mul(out=t1[:], in0=sig[:, B:2 * B], in1=c_old[:])
                nc.vector.tensor_mul(out=t2[:], in0=sig[:, 0:B], in1=cha[:])
                nc.vector.tensor_add(out=c_new[:], in0=t1[:], in1=t2[:])
            tch = p2_sb.tile([128, B], F32, name=f"tch{t}", tag="tch")
            nc.scalar.activation(out=tch[:], in_=c_new[:],
                                 func=mybir.ActivationFunctionType.Tanh)
            h16 = p2_sb.tile([128, B], BF16, name=f"h16_{t}", tag="h16")
            nc.vector.tensor_mul(out=h16[:], in0=sig[:, 2 * B:3 * B], in1=tch[:])
            nc.sync.dma_start(out=out_loc[t, :, :], in_=h16[:])
            if t + 1 < s_steps:
                nc.sync.dma_start(out=inb[t % 2][:], in_=h16[:])
                nc.gpsimd.collective_compute(
                    "AllGather", mybir.AluOpType.bypass,
                    replica_groups=[list(range(N_CORES))],
                    ins=[inb[t % 2][:].opt()], outs=[outb[t][:].opt()])
                nc.sync.dma_start(
                    out=hall[(t + 1) % 2][:].rearrange("p (k b) -> p k b", k=N_CORES),
                    in_=outb[t][:].rearrange("(k p) b -> p k b", p=128))
            else:
                nc.sync.dma_start(out=c_last[:, :], in_=c_new[:])

        next_tile = 0
        for nt in range(pref):
            emit_p1_tile(nt)
            next_tile = nt + 1
        for t in range(s_steps):
            emit_step(t)
            if t % CADENCE == CADENCE - 1 and next_tile < nt_total:
                emit_p1_tile(next_tile)
                next_tile += 1
        while next_tile < nt_total:
            emit_p1_tile(next_tile)
            next_tile += 1

    nc.compile()
    return nc


def _prep_inputs(input, Wx, Wh, b, s_steps=S):
    """Host-side reshape/cast into the per-core layouts."""
    bf = ml_dtypes.bfloat16
    # inT[d, s*B+b] = input[b, s, d]
    inT = np.ascontiguousarray(
        input[:, :s_steps, :].astype(bf).transpose(2, 1, 0)).reshape(D, s_steps * B)
    in_maps = []
    for j in range(N_CORES):
        lanes = slice(LPC * j, LPC * (j + 1))
        # wxT[d, g*LPC+l] = Wx[g, lanes, d]
        wxT = np.ascontiguousarray(
            Wx[:, lanes, :].astype(bf).transpose(2, 0, 1)).reshape(D, 4 * LPC)
        whT = np.ascontiguousarray(
            Wh[:, lanes, :].astype(bf).transpose(2, 0, 1)).reshape(H, 4 * LPC)
        bT = np.ascontiguousarray(b[:, lanes].astype(np.float32).T)  # [LPC, 4]
        in_maps.append({"inT": inT, "wxT": wxT, "whT": whT, "bT": bT})
    return in_maps


def _get_runner(s_steps=S):
    key = s_steps
    if key not in _cache:
        from runner import SpmdRunner
        nc = _build(s_steps)
        _cache[key] = SpmdRunner(nc, N_CORES)
    return _cache[key]


def kernel(input, Wx, Wh, b):
    input = np.asarray(input)
    Wx, Wh, b = np.asarray(Wx), np.asarray(Wh), np.asarray(b)
    r = _get_runner(S)
    in_maps = _prep_inputs(input, Wx, Wh, b, S)
    args = r.put_inputs(in_maps)
    outs = r.run(args)
    res = r.results(outs)
    # assemble: output[b, s, LPC*j+l] = out_loc[j][s, l, b]
    ol = np.stack([res[j]["out_loc"] for j in range(N_CORES)])  # [8, S, LPC, B] bf16
    output = np.ascontiguousarray(ol.transpose(3, 1, 0, 2)).reshape(B, S, H).astype(np.float32)
    cl = np.stack([res[j]["c_last"] for j in range(N_CORES)])   # [8, LPC, B] f32
    c_last = np.ascontiguousarray(cl.transpose(2, 0, 1)).reshape(B, H)
    h_last = output[:, -1, :].copy()
    return output, (h_last, c_last)


# revision 11
# speedup vs baseline: 1.1916x; 1.1916x over previous
"""LSTM forward on 8 Trainium2 NeuronCores (Bass/Tile).

Problem: B=64, S=512, D=H=1024, gates (i, f, o, c_hat):
    x_proj = einsum('bsd,ghd->sbgh', input, Wx)
    per step: g = x_proj[t] + h @ Wh[g]^T + b ; standard LSTM cell
    returns (output[B,S,H], (h_last, c_last))

Sharding: tensor-parallel over H. Core j owns hidden lanes [128j, 128j+128)
for all 4 gates. Per step, each core computes its 128 lanes of h_t and the
cores AllGather h_t (bf16, 16KB/core) for the next step's recurrent matmul.
The input projection (parallel over lanes too: each core needs only its own
Wx rows) is interleaved into the recurrence to fill the PE during AG waits.

Layout choice: the recurrent matmul puts gate-lanes on PSUM partitions
(lhsT = Wh^T chunk [128 h-lanes, 128 gate-lanes], rhs = gathered h^T chunk
[128 h-lanes, 64 batch]), so the whole epilogue runs on full 128 partitions
([lane, batch] tiles) and h_t^T [128, 64] comes out already in the layout the
AllGather needs — no transposes anywhere on the critical path.
"""
import numpy as np
import ml_dtypes
from contextlib import ExitStack

import concourse.bass as bass
import concourse.mybir as mybir
import concourse.tile as tile
from concourse import bacc

BF16 = mybir.dt.bfloat16
F32 = mybir.dt.float32

B, S, D, H = 64, 512, 1024, 1024
N_CORES = 8
LPC = H // N_CORES          # 128 lanes per core
KC = D // 128               # 8 contraction chunks
ROW_TILE = 512              # phase-1 rows (s,b) per tile = 8 steps
N_TILES = S * B // ROW_TILE  # 64
PREF_TILES = 16             # phase-1 tiles emitted before the recurrence
CADENCE = 9                 # then one tile every CADENCE steps

_cache = {}


def _build(s_steps=S):
    nt_total = s_steps * B // ROW_TILE
    pref = min(PREF_TILES, nt_total)
    nc = bacc.Bacc("TRN2", target_bir_lowering=False)

    inT = nc.dram_tensor("inT", [D, s_steps * B], BF16, kind="ExternalInput")
    wxT = nc.dram_tensor("wxT", [D, 4 * LPC], BF16, kind="ExternalInput")
    whT = nc.dram_tensor("whT", [H, 4 * LPC], BF16, kind="ExternalInput")
    bT = nc.dram_tensor("bT", [LPC, 4], F32, kind="ExternalInput")
    out_loc = nc.dram_tensor("out_loc", [s_steps, LPC, B], BF16, kind="ExternalOutput")
    c_last = nc.dram_tensor("c_last", [LPC, B], F32, kind="ExternalOutput")

    with ExitStack() as ctx:
        tc = ctx.enter_context(tile.TileContext(nc))
        singles = ctx.enter_context(tc.tile_pool(name="singles", bufs=1))
        p1_in = ctx.enter_context(tc.tile_pool(name="p1_in", bufs=2))
        p1_ps = ctx.enter_context(tc.tile_pool(name="p1_ps", bufs=2, space="PSUM"))
        p1_xp = ctx.enter_context(tc.tile_pool(name="p1_xp", bufs=3))
        p2_ps = ctx.enter_context(tc.tile_pool(name="p2_ps", bufs=2, space="PSUM"))
        p2_sb = ctx.enter_context(tc.tile_pool(name="p2_sb", bufs=2))
        p2_xp = ctx.enter_context(tc.tile_pool(name="p2_xp", bufs=4))
        dram = ctx.enter_context(tc.tile_pool(name="dram", bufs=1, space="DRAM"))

        # resident weights / bias
        wx_sb = singles.tile([128, KC * 4 * LPC], BF16, name="wx_sb")
        wh_sb = singles.tile([128, KC * 4 * LPC], BF16, name="wh_sb")
        bt_sb = singles.tile([LPC, 4], F32, name="bt_sb")
        nc.sync.dma_start(out=wx_sb[:].rearrange("p (k m) -> p k m", k=KC),
                          in_=wxT[:, :].rearrange("(k p) m -> p k m", p=128))
        nc.sync.dma_start(out=wh_sb[:].rearrange("p (k m) -> p k m", k=KC),
                          in_=whT[:, :].rearrange("(k p) m -> p k m", p=128))
        nc.sync.dma_start(out=bt_sb[:], in_=bT[:, :])

        # recurrence state; hall split per chunk so each matmul gates on the
        # smallest possible slice of the gathered h
        hall = [[p2_sb.tile([128, N_CORES * B // 2], BF16,
                            name=f"hall{i}_{h}", tag=f"hall{i}_{h}")
                 for h in range(2)] for i in range(2)]
        # ccomb[:, 0:B] = tanh(c_hat) staging, ccomb[:, B:2B] = c state
        ccomb = [p2_sb.tile([128, 2 * B], F32, name=f"ccomb{i}", tag=f"ccomb{i}")
                 for i in range(2)]

        # internal DRAM
        xp_d = dram.tile([s_steps, 128, 4 * B], F32, name="xp_d")
        inb = [dram.tile([128, B], BF16, name=f"inb{i}", tag=f"inb{i}")
               for i in range(2)]
        outb = [dram.tile([N_CORES * 128, B], BF16, name=f"outb{t}",
                          addr_space="Shared") for t in range(s_steps - 1)]

        def emit_p1_tile(nt):
            """Input projection for rows [512*nt, 512*nt+512) = steps 8nt..8nt+8."""
            r0 = ROW_TILE * nt
            in_tiles = []
            for k in range(KC):
                it = p1_in.tile([128, ROW_TILE], BF16, name=f"it{nt}_{k}", tag=f"it{k}")
                nc.sync.dma_start(out=it[:], in_=inT[128 * k:128 * (k + 1), r0:r0 + ROW_TILE])
                in_tiles.append(it)
            for g in range(4):
                ps = p1_ps.tile([128, ROW_TILE], F32, name=f"ps1_{nt}_{g}", tag="ps1")
                for k in range(KC):
                    nc.tensor.matmul(ps[:], lhsT=wx_sb[:, k * 512 + g * 128:k * 512 + (g + 1) * 128],
                                     rhs=in_tiles[k][:], start=(k == 0), stop=(k == KC - 1))
                xs = p1_xp.tile([128, ROW_TILE], F32, name=f"xs{nt}_{g}", tag="xs")
                nc.vector.tensor_scalar_add(out=xs[:], in0=ps[:], scalar1=bt_sb[:, g:g + 1])
                nc.sync.dma_start(
                    out=xp_d[8 * nt:8 * nt + 8, :, B * g:B * (g + 1)].rearrange("t p b -> p t b"),
                    in_=xs[:].rearrange("p (t b) -> p t b", t=8))

        def emit_step(t):
            xp2 = p2_xp.tile([128, 4 * B], F32, name=f"xp2_{t}", tag="xp2")
            nc.sync.dma_start(out=xp2[:], in_=xp_d[t, :, :])
            gbuf = p2_sb.tile([128, 4 * B], F32, name=f"g{t}", tag="gbuf")
            if t == 0:
                nc.vector.tensor_copy(out=gbuf[:], in_=xp2[:])
            else:
                ps = p2_ps.tile([128, 4 * B], F32, name=f"ps2_{t}", tag="ps2")
                h_in = hall[t % 2]
                for g in range(4):
                    for k in range(KC):
                        nc.tensor.matmul(
                            ps[:, B * g:B * (g + 1)],
                            lhsT=wh_sb[:, k * 512 + g * 128:k * 512 + (g + 1) * 128],
                            rhs=h_in[k // 4][:, B * (k % 4):B * (k % 4 + 1)],
                            start=(k == 0), stop=(k == KC - 1))
                nc.vector.tensor_add(out=gbuf[:], in0=ps[:], in1=xp2[:])
            # gates: i=[0:64] f=[64:128] o=[128:192] chat=[192:256]
            sig = p2_sb.tile([128, 3 * B], F32, name=f"sig{t}", tag="sig")
            cc_old, cc_new = ccomb[t % 2], ccomb[(t + 1) % 2]
            # (i, f) first and chat next: they gate the c-state chain; o later
            nc.scalar.activation(out=sig[:, 0:2 * B], in_=gbuf[:, 0:2 * B],
                                 func=mybir.ActivationFunctionType.Sigmoid)
            nc.scalar.activation(out=cc_old[:, 0:B], in_=gbuf[:, 3 * B:4 * B],
                                 func=mybir.ActivationFunctionType.Tanh)
            nc.scalar.activation(out=sig[:, 2 * B:3 * B], in_=gbuf[:, 2 * B:3 * B],
                                 func=mybir.ActivationFunctionType.Sigmoid)
            t12 = p2_sb.tile([128, 2 * B], F32, name=f"t12_{t}", tag="t12")
            if t == 0:
                # c_old = 0: c_new = i * chat
                nc.vector.tensor_mul(out=cc_new[:, B:2 * B], in0=sig[:, 0:B],
                                     in1=cc_old[:, 0:B])
            else:
                # (i | f) * (chat | c_old) in one op, then sum halves
                nc.vector.tensor_mul(out=t12[:], in0=sig[:, 0:2 * B], in1=cc_old[:])
                nc.vector.tensor_add(out=cc_new[:, B:2 * B], in0=t12[:, 0:B],
                                     in1=t12[:, B:2 * B])
            tch = p2_sb.tile([128, B], F32, name=f"tch{t}", tag="tch")
            nc.scalar.activation(out=tch[:], in_=cc_new[:, B:2 * B],
                                 func=mybir.ActivationFunctionType.Tanh)
            h16 = p2_sb.tile([128, B], BF16, name=f"h16_{t}", tag="h16")
            nc.vector.tensor_mul(out=h16[:], in0=sig[:, 2 * B:3 * B], in1=tch[:])
            if t + 1 < s_steps:
                nc.sync.dma_start(out=inb[t % 2][:], in_=h16[:])
                nc.gpsimd.collective_compute(
                    "AllGather", mybir.AluOpType.bypass,
                    replica_groups=[list(range(N_CORES))],
                    ins=[inb[t % 2][:].opt()], outs=[outb[t][:].opt()])
                nxt = hall[(t + 1) % 2]
                for h in range(2):
                    nc.sync.dma_start(
                        out=nxt[h][:].rearrange("p (k b) -> p k b", k=N_CORES // 2),
                        in_=outb[t][128 * 4 * h:128 * 4 * (h + 1), :]
                        .rearrange("(k p) b -> p k b", p=128))
            else:
                nc.sync.dma_start(out=c_last[:, :], in_=cc_new[:, B:2 * B])
            nc.scalar.dma_start(out=out_loc[t, :, :], in_=h16[:])

        next_tile = 0
        for nt in range(pref):
            emit_p1_tile(nt)
            next_tile = nt + 1
        for t in range(s_steps):
            emit_step(t)
            if t % CADENCE == CADENCE - 1 and next_tile < nt_total:
                emit_p1_tile(next_tile)
                next_tile += 1
        while next_tile < nt_total:
            emit_p1_tile(next_tile)
            next_tile += 1

    nc.compile()
    return nc


def _prep_inputs(input, Wx, Wh, b, s_steps=S):
    """Host-side reshape/cast into the per-core layouts."""
    bf = ml_dtypes.bfloat16
    # inT[d, s*B+b] = input[b, s, d]
    inT = np.ascontiguousarray(
        input[:, :s_steps, :].astype(bf).transpose(2, 1, 0)).reshape(D, s_steps * B)
    in_maps = []
    for j in range(N_CORES):
        lanes = slice(LPC * j, LPC * (j + 1))
        # wxT[d, g*LPC+l] = Wx[g, lanes, d]
        wxT = np.ascontiguousarray(
            Wx[:, lanes, :].astype(bf).transpose(2, 0, 1)).reshape(D, 4 * LPC)
        whT = np.ascontiguousarray(
            Wh[:, lanes, :].astype(bf).transpose(2, 0, 1)).reshape(H, 4 * LPC)
        bT = np.ascontiguousarray(b[:, lanes].astype(np.float32).T)  # [LPC, 4]
        in_maps.append({"inT": inT, "wxT": wxT, "whT": whT, "bT": bT})
    return in_maps


class SpmdRunner:
    """Jit once, execute many times (mirrors bass2jax.run_bass_via_pjrt)."""

    def __init__(self, nc, n_cores):
        import jax
        from jax.sharding import Mesh, PartitionSpec
        from jax.experimental.shard_map import shard_map
        from concourse.bass2jax import (
            _bass_exec_p, partition_id_tensor, install_neuronx_cc_hook)

        install_neuronx_cc_hook()
        self.jax = jax
        self.PartitionSpec = PartitionSpec
        self.nc = nc
        self.n_cores = n_cores
        partition_name = nc.partition_id_tensor.name if nc.partition_id_tensor else None
        in_names, out_names, out_avals, zero_outs = [], [], [], []
        for alloc in nc.m.functions[0].allocations:
            if not isinstance(alloc, mybir.MemoryLocationSet):
                continue
            name = alloc.memorylocations[0].name
            if alloc.kind == "ExternalInput":
                if name != partition_name:
                    in_names.append(name)
            elif alloc.kind == "ExternalOutput":
                out_names.append(name)
                shape = tuple(alloc.tensor_shape)
                dtype = mybir.dt.np(alloc.dtype)
                out_avals.append(jax.core.ShapedArray(shape, dtype))
                zero_outs.append(np.zeros(shape, dtype))
        self.in_names, self.out_names = in_names, out_names
        self.zero_outs = zero_outs
        self.n_params = len(in_names)
        n_outs = len(out_avals)
        all_in_names = in_names + out_names
        if partition_name is not None:
            all_in_names.append(partition_name)

        def _body(*args):
            operands = list(args)
            if partition_name is not None:
                operands.append(partition_id_tensor())
            return tuple(_bass_exec_p.bind(
                *operands,
                out_avals=tuple(out_avals),
                in_names=tuple(all_in_names),
                out_names=tuple(out_names),
                lowering_input_output_aliases=(),
                sim_require_finite=True,
                sim_require_nnan=True,
                nc=nc,
            ))

        devices = jax.devices()[:n_cores]
        self.mesh = Mesh(np.asarray(devices), ("core",))
        in_specs = (PartitionSpec("core"),) * (self.n_params + n_outs)
        out_specs = (PartitionSpec("core"),) * n_outs
        self.fn = jax.jit(
            shard_map(_body, mesh=self.mesh, in_specs=in_specs,
                      out_specs=out_specs, check_rep=False),
            keep_unused=True,
        )

    def put_inputs(self, in_maps):
        jax = self.jax
        sharding = jax.sharding.NamedSharding(self.mesh, self.PartitionSpec("core"))
        args = []
        for i in range(self.n_params):
            glob = np.concatenate(
                [np.asarray(m[self.in_names[i]]) for m in in_maps], axis=0)
            args.append(jax.device_put(glob, sharding))
        for z in self.zero_outs:
            args.append(jax.device_put(np.concatenate([z] * self.n_cores, axis=0),
                                       sharding))
        return args

    def run(self, args):
        outs = self.fn(*args)
        self.jax.block_until_ready(outs)
        return outs

    def results(self, outs):
        res = []
        for c in range(self.n_cores):
            d = {}
            for i, name in enumerate(self.out_names):
                shape = self.zero_outs[i].shape
                d[name] = np.asarray(outs[i]).reshape(self.n_cores, *shape)[c]
            res.append(d)
        return res


def _get_runner(s_steps=S):
    key = s_steps
    if key not in _cache:
        nc = _build(s_steps)
        _cache[key] = SpmdRunner(nc, N_CORES)
    return _cache[key]


def kernel(input, Wx, Wh, b):
    input = np.asarray(input)
    Wx, Wh, b = np.asarray(Wx), np.asarray(Wh), np.asarray(b)
    r = _get_runner(S)
    in_maps = _prep_inputs(input, Wx, Wh, b, S)
    args = r.put_inputs(in_maps)
    outs = r.run(args)
    res = r.results(outs)
    # assemble: output[b, s, LPC*j+l] = out_loc[j][s, l, b]
    ol = np.stack([res[j]["out_loc"] for j in range(N_CORES)])  # [8, S, LPC, B] bf16
    output = np.ascontiguousarray(ol.transpose(3, 1, 0, 2)).reshape(B, S, H).astype(np.float32)
    cl = np.stack([res[j]["c_last"] for j in range(N_CORES)])   # [8, LPC, B] f32
    c_last = np.ascontiguousarray(cl.transpose(2, 0, 1)).reshape(B, H)
    h_last = output[:, -1, :].copy()
    return output, (h_last, c_last)
